# revision 48
# baseline (speedup 1.0000x reference)
"""Gated Linear Attention on 8 Trainium2 NeuronCores.

Sharding: one (batch, head) pair per core (B=2 x H=4 = 8 cores). Each core
computes its head's full pipeline and emits a partial [N, D] output (bf16);
the host sums the 4 head partials per batch in f32.

v4 design:
  * All heavy matmuls in bf16 (1 PE cycle/row vs 4 for fp32); PSUM accums f32.
  * Per-chunk LOCAL decay (no global cumsum carry chain): within chunk c,
    b = L^T g'' (local inclusive cumsum). q~=q*exp(-b/16), k~=k*exp(+b/16);
    cross-chunk state rescaled once per chunk by the per-feature factor
    f = exp(-b_last/16) = ET[:,last]:  W_c = diag(f) (W_{c-1} + k~^T v).
    Local exponent args <= ~6, safe in bf16/f32.
  * z-projection folded into the main projection blob. Projection emission is
    split bank1-first (gate|z) so the per-chunk softplus (2 ACT + 1 DVE ops,
    clamp folded into ln via min(u,e^48)) overlaps the qkv matmuls and the
    L-matmul never stalls the PE.
  * ACT table discipline: exp+ln resolve to the combined table by blanking
    the exp-only/ln-only sets for the load-insertion pass (ids still index
    the real act_info.json). Silu via tanh in the final phase. 2 loads total.
  * Engine balance: Pool/GpSimd takes the psum->sbuf eviction copies
    (at-mask, state, ssq, oT); per-queue semaphore overhead (~250ns/op on
    ACT/DVE) is minimized by merging adjacent-column copies (q|k, v|gate)
    and batching phase-D tanh over all chunks in one instruction.
  * RMS r deferred and folded into the silu gate; bf16 I/O; contiguous 2KB+
    DMA rows; DMA descriptor issues spread across idle engine queues.
"""

import os
from contextlib import ExitStack

import numpy as np
import ml_dtypes

import concourse.bass as bass
import concourse.tile as tile
from concourse import bacc, mybir
from concourse.tile_rust import add_dep_helper
from concourse.bass_utils import run_bass_kernel_spmd

F32 = mybir.dt.float32
BF16 = mybir.dt.bfloat16
AF = mybir.ActivationFunctionType
ALU = mybir.AluOpType

B, N, D, H = 2, 1024, 1024, 4
KD, VD, DK, DV = 512, 1024, 128, 256
C = 128                    # chunk length (= token partitions)
NCH = N // C               # 8 chunks
NK = D // 128              # 8 contraction tiles
BLOBW = 896                # blob cols: q128 | k128 | v256 | gate256 | z128
GLN = 16.0
EPS = 1e-5
E48 = float(np.exp(48.0).astype(np.float32))

# module-level stash so test.py can grab profiling results
LAST_RESULTS = None

_BLANK_TABLES = ("exp_and_others", "natural_log", "exp_and_friends")
_tables_patched = False


def _patch_act_tables():
    """Steer the ACT-table-load chooser toward natural_log_exp_and_others so
    exp+ln never alternate table loads. Only the (name -> funcs) map used by
    the load-insertion pass and CoreSim is filtered; emitted act_func_set_ids
    still index the real act_info.json, so walrus/hardware see valid sets."""
    global _tables_patched
    if _tables_patched:
        return
    _tables_patched = True
    from concourse import hw_specs, bass_interp
    orig = hw_specs.get_activation_tables

    def patched(arch):
        tabs = dict(orig(arch))
        for name in _BLANK_TABLES:
            if name in tabs:
                tabs[name] = set()
        return tabs

    bacc.get_activation_tables = patched
    bass_interp.get_activation_tables = patched



def _emit_kernel(ctx: ExitStack, tc: "tile.TileContext", ap: dict):
    nc = tc.nc

    # Chain all PE instructions in program order (PE executes in-order; this
    # keeps the Tile scheduler from reordering matmuls within a PSUM bank,
    # which would break has_written clear ordering).
    pe_prev = [None]

    def mm(*args, **kw):
        inst = nc.tensor.matmul(*args, **kw)
        if kw.get("skip_group_check") or kw.get("start") in (False, None):
            # keep explicit order only for matmuls that join open psum groups
            if pe_prev[0] is not None:
                add_dep_helper(inst.ins, pe_prev[0], sync=False,
                               reason="pe-order")
        pe_prev[0] = inst.ins
        return inst

    def tr_(out, in_, ident):
        inst = nc.tensor.transpose(out, in_, ident)
        if pe_prev[0] is not None:
            add_dep_helper(inst.ins, pe_prev[0], sync=False, reason="pe-order")
        pe_prev[0] = inst.ins
        return inst

    xT, wblob, woutT = ap["xT"], ap["wblob"], ap["woutT"]
    bgk2, lmask, lmaskb = ap["bgk2"], ap["lmask"], ap["lmaskb"]
    ident32, identb = ap["ident32"], ap["identb"]
    out = ap["out"]

    consts = ctx.enter_context(tc.tile_pool(name="consts", bufs=1))
    wpool = ctx.enter_context(tc.tile_pool(name="wpool", bufs=1))
    work = ctx.enter_context(tc.tile_pool(name="work", bufs=2))
    wst = ctx.enter_context(tc.tile_pool(name="wst", bufs=2))
    outp = ctx.enter_context(tc.tile_pool(name="outp", bufs=2))
    ppool = ctx.enter_context(tc.tile_pool(name="ppool", bufs=2, space="PSUM"))
    ptr = ctx.enter_context(tc.tile_pool(name="ptr", bufs=1, space="PSUM"))
    pbf = ctx.enter_context(tc.tile_pool(name="pbf", bufs=2, space="PSUM"))
    pao = ctx.enter_context(tc.tile_pool(name="pao", bufs=1, space="PSUM"))

    # bgk2 first on the scalar ring (tiny; needed by chunk 0's bias matmul)
    bg_sb = consts.tile([1, 128], BF16)
    nc.scalar.dma_start(out=bg_sb[:], in_=bgk2[:])
    ones_row = consts.tile([1, 128], BF16)
    nc.vector.memset(ones_row[:], 1.0)
    ones_col = consts.tile([128, 1], BF16)
    nc.vector.memset(ones_col[:], 1.0)
    eps_sb = consts.tile([128, 1], F32)
    nc.vector.memset(eps_sb[:], EPS)

    # ---- weights + x (bf16): x chunk 0 first (gates the first matmul), blob
    # on the gpsimd queue, rest of x on sync, consts on vector. Each
    # dma_start costs ~600ns of issue time on its queue, so spread them.
    # need-ordered issue across the three ~100GB/s DMA rings: x0 + the whole
    # weight blob gate chunk 0, so x0 leads the gpsimd ring and wb tiles
    # rotate across all three rings (all land ~15us vs 26us on one ring);
    # later x chunks follow on sync/scalar behind their wb shares.
    xsb = wpool.tile([128, N, NK], BF16)
    wb_sb = wpool.tile([128, NK, BLOBW], BF16)
    nc.gpsimd.dma_start(out=xsb[:, 0:C, :], in_=xT[:, 0:C, :])
    wq = [nc.sync, nc.scalar, nc.gpsimd]
    for k in range(NK):
        wq[k % 3].dma_start(out=wb_sb[:, k, :], in_=wblob[k])
    nc.gpsimd.dma_start(out=xsb[:, C:2 * C, :], in_=xT[:, C:2 * C, :])
    for c in range(2, NCH):
        lo = c * C
        q = nc.sync if c % 2 == 0 else nc.scalar
        q.dma_start(out=xsb[:, lo:lo + C, :], in_=xT[:, lo:lo + C, :])
    wout_sb = wpool.tile([128, 2, D], BF16)
    for j in range(2):
        nc.gpsimd.dma_start(out=wout_sb[:, j, :], in_=woutT[j])

    # remaining constants after the weight-blob shares on the scalar ring
    # (need order: Lb by the first L-matmul ~16us, idb by the transposes ~18,
    # L by the at-mask ~20)
    Lb_sb = consts.tile([128, 128], BF16)
    nc.scalar.dma_start(out=Lb_sb[:], in_=lmaskb[:])
    idb_sb = consts.tile([128, 128], BF16)
    nc.scalar.dma_start(out=idb_sb[:], in_=identb[:])
    L_sb = consts.tile([128, 128], F32)          # L[s,t]=1 iff s<=t (triu)
    nc.scalar.dma_start(out=L_sb[:], in_=lmask[:])


    # PE clock warmup: dummy matmuls while the first DMAs land. The tensor
    # engine needs ~3us of continuous execution to reach max frequency; these
    # fill the DMA wait so the real chunks run at full speed from the start.
    warm = consts.tile([128, 512], BF16)
    nc.vector.memset(warm[:], 0.0)
    wps = pao.tile([128, 512], F32, tag="big")
    for i in range(13):
        mm(wps[:], lhsT=warm[:, 0:128], rhs=warm[:],
           start=(i == 0), stop=(i == 12))

    # ---- main loop ---------------------------------------------------------
    # proj psum [128,1024]: bank0 {q 0:128 | k 128:256 | v 256:512}
    # bank1 {gate 512:768 | z 768:896 | b_loc 896:1024}. bank1 (and its bias
    # close) is emitted BEFORE bank0 so softplus overlaps the qkv matmuls.
    # b (token-major) and bT (feature-major) are both produced directly by
    # matmuls against the triangular mask (b = L^T g, bT = g^T L).
    # The ENTIRE output path (silu gate via reciprocal_approx_fast - no
    # activation-table switch - RMS scale, final projection, store) is inlined
    # per chunk, one chunk behind the front of the pipeline, so outputs
    # stream to HBM throughout the loop and no drain phase remains.

    def P1(c):
        proj = ppool.tile([128, 1024], F32, tag="proj")
        tok = slice(c * C, (c + 1) * C)
        for k in range(NK):
            mm(proj[:, 512:896], lhsT=xsb[:, tok, k], rhs=wb_sb[:, k, 512:896],
               start=(k == 0), stop=False)
        bias_mm = mm(proj[:, 768:896], lhsT=ones_row[:], rhs=bg_sb[:],
                     start=False, stop=True)
        # softplus part a: e1 = exp(-z)
        e1 = work.tile([128, 128], F32, tag="e1")
        i = nc.scalar.activation(e1[:], proj[:, 768:896], AF.Exp, scale=-1.0)
        add_dep_helper(i.ins, bias_mm.ins, sync=False, reason="z after close")
        return proj, e1

    def SPb(c, e1):
        u1 = work.tile([128, 128], F32, tag="u1")
        nc.vector.tensor_scalar(u1[:], e1[:], 1.0, E48, ALU.add, ALU.min)
        return u1

    def SPc(c, u1):
        g_c = work.tile([128, 128], BF16, tag="g")
        nc.scalar.activation(g_c[:], u1[:], AF.Ln)
        return g_c

    def P0(c, proj):
        tok = slice(c * C, (c + 1) * C)
        for k in range(NK):
            mm(proj[:, 0:512], lhsT=xsb[:, tok, k], rhs=wb_sb[:, k, 0:512],
               start=(k == 0), stop=(k == NK - 1))

    def Bmm(c, proj, g_c):
        bmm = mm(proj[:, 896:1024], lhsT=Lb_sb[:], rhs=g_c[:],
                 start=False, stop=False, skip_group_check=True)
        sml = ptr.tile([128, 512], F32, tag="sml")   # bT | at | ssq
        mm(sml[:, 0:128], lhsT=g_c[:], rhs=Lb_sb[:], start=True, stop=True)
        return sml, bmm

    def Ex(c, proj, sml, bmm):
        En_tok = work.tile([128, 128], BF16, tag="Ent")
        i = nc.scalar.activation(En_tok[:], proj[:, 896:1024], AF.Exp,
                                 scale=1.0 / GLN)
        add_dep_helper(i.ins, bmm.ins, sync=False, reason="b after b-mm")
        ET = work.tile([128, 128], BF16, tag="ET")
        nc.scalar.activation(ET[:], sml[:, 0:128], AF.Exp, scale=-1.0 / GLN)
        EnT = work.tile([128, 128], BF16, tag="EnT")
        nc.scalar.activation(EnT[:], sml[:, 0:128], AF.Exp, scale=1.0 / GLN)
        f_vec = work.tile([128, 1], F32, tag="f")
        nc.scalar.activation(f_vec[:], sml[:, 127:128], AF.Exp, scale=-1.0 / GLN)
        # silu ingredient: eg = exp(-ug) straight from psum
        eg = work.tile([128, DV], F32, tag="eg")
        nc.scalar.activation(eg[:], proj[:, 512:768], AF.Exp, scale=-1.0)
        return En_tok, ET, EnT, f_vec, eg

    def QK(c, proj):
        qk_sb = work.tile([128, 256], BF16, tag="qk")
        nc.vector.tensor_copy(qk_sb[:], proj[:, 0:256])
        v_tm = work.tile([128, DV], BF16, tag="v")
        nc.scalar.copy(v_tm[:], proj[:, 256:512])
        ug = work.tile([128, DV], F32, tag="ug")
        nc.scalar.copy(ug[:], proj[:, 512:768])
        return qk_sb, v_tm, ug

    def T(c, qk_sb):
        tq = pbf.tile([128, 256], BF16, tag="tqk")
        tr_(tq[:, 0:128], qk_sb[:, 0:128], idb_sb[:])
        tr_(tq[:, 128:256], qk_sb[:, 128:256], idb_sb[:])
        return tq

    def M(c, tq, qk_sb, En_tok, ET, EnT, eg):
        qtT = work.tile([128, 128], BF16, tag="qtT")
        nc.vector.tensor_mul(qtT[:], tq[:, 0:128], ET[:])
        ktT = work.tile([128, 128], BF16, tag="ktT")
        nc.vector.tensor_mul(ktT[:], tq[:, 128:256], EnT[:])
        kt_tm = work.tile([128, 128], BF16, tag="kt")
        nc.vector.tensor_mul(kt_tm[:], qk_sb[:, 128:256], En_tok[:])
        # silu: rf = 1/(1+eg)
        dg = work.tile([128, DV], F32, tag="dg")
        nc.vector.tensor_scalar_add(dg[:], eg[:], 1.0)
        rf = work.tile([128, DV], F32, tag="rf")
        nc.vector.reciprocal_approx_fast(rf[:], dg[:])
        return qtT, ktT, kt_tm, rf

    def A(c, sml, qtT, ktT):
        mm(sml[:, 128:256], lhsT=ktT[:], rhs=qtT[:], start=True, stop=True)

    def AM(c, sml):
        at_m = work.tile([128, 128], BF16, tag="atm")
        nc.vector.tensor_mul(at_m[:], sml[:, 128:256], L_sb[:])
        return at_m

    def OT(c, at_m, qtT, v_tm):
        big = pao.tile([128, 512], F32, tag="big")
        ot = big[:, 0:256]
        if c > 0:
            w_prev = state["w_prev_for_o"]
            mm(ot[:, 0:128], lhsT=w_prev[:, 0:128], rhs=qtT[:],
               start=True, stop=False)
            mm(ot[:, 128:256], lhsT=w_prev[:, 128:256], rhs=qtT[:],
               start=False, stop=False, skip_group_check=True)
            mm(ot[:, 0:128], lhsT=v_tm[:, 0:128], rhs=at_m[:],
               start=False, stop=False, skip_group_check=True)
        else:
            mm(ot[:, 0:128], lhsT=v_tm[:, 0:128], rhs=at_m[:],
               start=True, stop=False)
        mm(ot[:, 128:256], lhsT=v_tm[:, 128:256], rhs=at_m[:],
           start=False, stop=False, skip_group_check=True)
        return big

    def ST(c, big, kt_tm, v_tm, f_vec):
        st = big[:, 256:512]
        mm(st[:], lhsT=kt_tm[:], rhs=v_tm[:], start=True, stop=False,
           skip_group_check=True)
        if c > 0:
            mm(st[:], lhsT=idb_sb[:], rhs=state["w_prev"][:], start=False,
               stop=False, skip_group_check=True)
        if c < NCH - 1:
            w_new = wst.tile([128, DV], BF16, tag="w")
            nc.vector.tensor_scalar(w_new[:], st[:], f_vec[:], None, ALU.mult)
            state["w_prev"] = w_new

    def SQ(c, big):
        sq = work.tile([128, DV], BF16, tag="sq")
        nc.scalar.square(sq[:], big[:, 0:256])
        return sq

    def SSQ(c, sq, sml):
        ssq = sml[:, 256:257]
        mm(ssq, lhsT=sq[:, 0:128], rhs=ones_col[:],
           start=True, stop=False, skip_group_check=True)
        mm(ssq, lhsT=sq[:, 128:256], rhs=ones_col[:],
           start=False, stop=False, skip_group_check=True)
        return ssq

    def R(c, ssq):
        s_c = work.tile([128, 1], F32, tag="s")
        nc.scalar.activation(s_c[:], ssq, AF.Ln, scale=1.0 / DV, bias=eps_sb[:])
        r_c = work.tile([128, 1], F32, tag="r")
        nc.scalar.activation(r_c[:], s_c[:], AF.Exp, scale=-0.5)
        return r_c

    def GATE(c, ug, rf, r_c):
        # gate*r = (ug*r) * sigmoid(ug), sigmoid via fast reciprocal
        gate_tm = work.tile([128, DV], BF16, tag="gate")
        nc.vector.scalar_tensor_tensor(gate_tm[:], ug[:], r_c[:], rf[:],
                                       ALU.mult, ALU.mult)
        return gate_tm

    def TR2(c, gate_tm):
        tr2 = pbf.tile([128, 256], BF16, tag="tqk")
        tr_(tr2[:, 0:128], gate_tm[:, 0:128], idb_sb[:])
        tr_(tr2[:, 128:256], gate_tm[:, 128:256], idb_sb[:])
        return tr2

    def OG(c, tr2, big):
        gateT = work.tile([128, DV], F32, tag="gT")
        nc.scalar.copy(gateT[:], tr2[:])
        og = work.tile([128, DV], BF16, tag="og")
        nc.vector.tensor_mul(og[:], big[:, 0:256], gateT[:])
        return og

    def FIN(c, og):
        tok = slice(c * C, (c + 1) * C)
        fin = ppool.tile([128, 1024], F32, tag="proj")
        for nb in range(2):
            cols = slice(nb * 512, (nb + 1) * 512)
            mm(fin[:, cols], lhsT=og[:, 0:128],
               rhs=wout_sb[:, 0, cols], start=True, stop=False)
            mm(fin[:, cols], lhsT=og[:, 128:256],
               rhs=wout_sb[:, 1, cols], start=False, stop=True)
        o_sb = outp.tile([128, 1024], BF16, tag="o")
        nc.vector.tensor_copy(o_sb[:, 0:512], fin[:, 0:512])
        nc.vector.tensor_copy(o_sb[:, 512:1024], fin[:, 512:1024])
        nc.sync.dma_start(out=out[tok, :], in_=o_sb[:])

    # ---- pipeline driver ----
    state = {"w_prev": None, "w_prev_for_o": None}
    pend = {}
    prv = None   # chunk c-1's (ug, rf, big, r) for the interleaved output tail

    proj0, e1_0 = P1(0)
    u1_0 = SPb(0, e1_0)
    g_0 = SPc(0, u1_0)
    P0(0, proj0)
    pend[0] = dict(proj=proj0, g=g_0)

    for c in range(NCH):
        p = pend[c]
        proj, g_c = p["proj"], p["g"]
        if prv is not None:
            r_p = R(c - 1, prv["ssq"])
        sml, bmm = Bmm(c, proj, g_c)
        En_tok, ET, EnT, f_vec, eg = Ex(c, proj, sml, bmm)
        qk_sb, v_tm, ug = QK(c, proj)
        if prv is not None:
            gate_p = GATE(c - 1, prv["ug"], prv["rf"], r_p)
        if c + 1 < NCH:
            projn, e1n = P1(c + 1)
        tq = T(c, qk_sb)
        if prv is not None:
            tr2_p = TR2(c - 1, gate_p)
        qtT, ktT, kt_tm, rf = M(c, tq, qk_sb, En_tok, ET, EnT, eg)
        if prv is not None:
            og_p = OG(c - 1, tr2_p, prv["big"])
        if c + 1 < NCH:
            u1n = SPb(c + 1, e1n)
            P0(c + 1, projn)
            gn = SPc(c + 1, u1n)
            pend[c + 1] = dict(proj=projn, g=gn)
        A(c, sml, qtT, ktT)
        at_m = AM(c, sml)
        if prv is not None:
            FIN(c - 1, og_p)
        state["w_prev_for_o"] = state["w_prev"]
        big = OT(c, at_m, qtT, v_tm)
        sq = SQ(c, big)
        ST(c, big, kt_tm, v_tm, f_vec)
        ssq_c = SSQ(c, sq, sml)
        prv = dict(ug=ug, rf=rf, big=big, ssq=ssq_c)

    # last chunk's output tail
    r_p = R(NCH - 1, prv["ssq"])
    gate_p = GATE(NCH - 1, prv["ug"], prv["rf"], r_p)
    tr2_p = TR2(NCH - 1, gate_p)
    og_p = OG(NCH - 1, tr2_p, prv["big"])
    FIN(NCH - 1, og_p)

def _build_nc():
    _patch_act_tables()
    nc = bacc.Bacc("TRN2", target_bir_lowering=False, debug=False, num_devices=8)
    ap = {
        "xT": nc.dram_tensor("xT", [128, N, NK], BF16, kind="ExternalInput").ap(),
        "wblob": nc.dram_tensor("wblob", [NK, 128, BLOBW], BF16,
                                kind="ExternalInput").ap(),
        "woutT": nc.dram_tensor("woutT", [2, 128, D], BF16,
                                kind="ExternalInput").ap(),
        "bgk2": nc.dram_tensor("bgk2", [1, 128], BF16, kind="ExternalInput").ap(),
        "lmask": nc.dram_tensor("lmask", [128, 128], F32,
                                kind="ExternalInput").ap(),
        "lmaskb": nc.dram_tensor("lmaskb", [128, 128], BF16,
                                 kind="ExternalInput").ap(),
        "ident32": nc.dram_tensor("ident32", [128, 128], F32,
                                  kind="ExternalInput").ap(),
        "identb": nc.dram_tensor("identb", [128, 128], BF16,
                                 kind="ExternalInput").ap(),
        "out": nc.dram_tensor("out", [N, D], BF16, kind="ExternalOutput").ap(),
    }
    with tile.TileContext(nc) as tc:
        with ExitStack() as ctx:
            _emit_kernel(ctx, tc, ap)
    nc.compile()
    return nc


def kernel(x, Wq, Wk, Wv, Wg, Wgk1, Wgk2, bgk2, Wout, rms_w):
    global LAST_RESULTS
    BF = ml_dtypes.bfloat16
    x = np.asarray(x, np.float32)
    Wz = (np.asarray(Wgk1, np.float32) @ np.asarray(Wgk2, np.float32))
    L = np.triu(np.ones((C, C), np.float32))
    I32 = np.eye(128, dtype=np.float32)

    in_maps = []
    for core in range(8):
        b, h = core // H, core % H
        xTb = np.ascontiguousarray(
            x[b].T.reshape(NK, 128, N).transpose(1, 2, 0)).astype(BF)
        blob = np.ascontiguousarray(np.concatenate([
            Wq[:, h * DK:(h + 1) * DK], Wk[:, h * DK:(h + 1) * DK],
            Wv[:, h * DV:(h + 1) * DV], Wg[:, h * DV:(h + 1) * DV],
            Wz[:, h * DK:(h + 1) * DK]],
            axis=1).astype(np.float32)).reshape(NK, 128, BLOBW).astype(BF)
        woutP = np.ascontiguousarray(
            (np.asarray(rms_w, np.float32)[:, None]
             * np.asarray(Wout, np.float32)[h * DV:(h + 1) * DV])
        ).reshape(2, 128, D).astype(BF)
        in_maps.append({
            "xT": xTb,
            "wblob": blob,
            "woutT": woutP,
            "bgk2": np.ascontiguousarray(
                np.asarray(bgk2, np.float32)[h * DK:(h + 1) * DK][None, :]
            ).astype(BF),
            "lmask": L,
            "lmaskb": L.astype(BF),
            "ident32": I32,
            "identb": I32.astype(BF),
        })

    nc = _build_nc()
    trace = os.environ.get("BASSGLA_TRACE", "0") == "1"
    res = run_bass_kernel_spmd(nc, in_maps, list(range(8)), trace=trace)
    LAST_RESULTS = res

    out = np.zeros((B, N, D), np.float32)
    for core in range(8):
        out[core // H] += np.asarray(res.results[core]["out"], np.float32)
    return out
